# revision 2
# baseline (speedup 1.0000x reference)
"""Trainium2 Bass kernel for y = x*x - 1 (elementwise, f32).

Full input x: (8192, 16384) f32. Sharded row-wise across 8 NeuronCores
(data parallel, no communication): each core processes a (1024, 16384)
slice. Memory-bound: per core 64 MiB in + 64 MiB out at the ~358 GB/s
per-core HBM limit => ~375 us floor; measured ~400 us/pass steady state.

Per-core pipeline (Tile-scheduled): 8 row-block tiles of [128, 16384]
(8 MiB, fully contiguous in DRAM => maximally efficient DMA descriptors),
double-buffered: HWDGE DMA load -> ScalarE Square (in-place) -> VectorE
tensor_scalar add -1 (in-place, 2x mode for f32 SBUF) -> HWDGE DMA store.
Both compute engines run far under the DMA roofline, so DMA stays the
bottleneck.

Swept alternatives (K-pass For_i loop-slope, median of 12 calls,
K=192 vs 576; reproducible to ~±2 us): this config = 393-395 us
(341 GB/s/core combined), equal to the measured PURE-READ per-core
ceiling (337-344 GB/s; load-only probes at f16384/b2, f8192/b4,
f4096/b8), i.e. HBM is saturated and there is no pipeline slack.
Every perturbation degrades it:
  bufs=3 (same tile) 415 us; f8192/b2 411, b3 428, b4 425, b5 421;
  f4096/b8 408; store on scalar ring 415; load via gpsimd/SWDGE 405;
  compute swapped DVE-square+ACT-add 412; stores chunked in halves
  422; last-tile split in quarters 433; compute removed entirely 424.
Mechanism: deeper buffering / finer store granularity raises the
number of concurrently interleaved HBM read+write streams (R/W
turnaround penalty), while the two-deep pipeline keeps ~1 load + 1
store in flight with compute latency desynchronizing the two buffer
chains so per-DMA issue/receipt gaps (~3 us) stay hidden.
"""

import sys

import numpy as np

if "/opt/trn_rl_repo" not in sys.path:
    sys.path.insert(0, "/opt/trn_rl_repo")

M, N = 8192, 16384
N_CORES = 8
ROWS_PER_CORE = M // N_CORES  # 1024
P = 128  # SBUF partitions
FREE = 16384  # tile free-dim elements (8 MiB f32 tiles, contiguous rows)
BUFS = 2

_nc_cache = {}


def _build():
    key = (ROWS_PER_CORE, N, FREE, BUFS)
    if key in _nc_cache:
        return _nc_cache[key]

    import concourse.mybir as mybir
    from concourse import bacc
    from concourse.tile import TileContext

    # Bacc (not plain Bass): its finalize() runs generate_event_semaphores,
    # which splits multi-semaphore waits into standalone event instructions.
    # Raw Bass modules with >1 wait on a DMA fail walrus codegen ("Too many
    # sync wait commands").
    nc = bacc.Bacc("TRN2")
    x = nc.dram_tensor(
        "x", [ROWS_PER_CORE, N], mybir.dt.float32, kind="ExternalInput"
    )
    y = nc.dram_tensor(
        "y", [ROWS_PER_CORE, N], mybir.dt.float32, kind="ExternalOutput"
    )
    xv = x.rearrange("(n p) m -> n p m", p=P)  # [8, 128, 16384]
    yv = y.rearrange("(n p) m -> n p m", p=P)
    n_blocks = ROWS_PER_CORE // P
    n_f = N // FREE

    with TileContext(nc) as tc:
        with tc.tile_pool(name="buf", bufs=BUFS) as pool:
            for nb in range(n_blocks):
                for f in range(n_f):
                    t = pool.tile([P, FREE], mybir.dt.float32)
                    src = xv[nb, :, f * FREE : (f + 1) * FREE]
                    dst = yv[nb, :, f * FREE : (f + 1) * FREE]
                    nc.sync.dma_start(t[:], src)
                    nc.scalar.activation(
                        t[:], t[:], mybir.ActivationFunctionType.Square
                    )
                    nc.vector.tensor_scalar_add(t[:], t[:], -1.0)
                    nc.sync.dma_start(dst, t[:])

    if not nc.is_finalized():
        nc.finalize()
    _nc_cache[key] = nc
    return nc


def kernel(x):
    from concourse.bass_utils import run_bass_kernel_spmd

    x = np.ascontiguousarray(np.asarray(x, dtype=np.float32))
    assert x.shape == (M, N), x.shape

    nc = _build()
    shards = np.split(x, N_CORES, axis=0)
    in_maps = [{"x": s} for s in shards]
    res = run_bass_kernel_spmd(nc, in_maps, core_ids=list(range(N_CORES)))
    out = np.concatenate([r["y"] for r in res.results], axis=0)
    return out.astype(np.float32, copy=False)

